# revision 7
# baseline (speedup 1.0000x reference)
"""Trainium2 Bass kernel for nn_HGC_SCN (2x 3-layer GCN + projection + path attention).

Strategy (8 NeuronCores, SPMD):
  - Nodes sharded across cores (6272 padded rows per core incl. 22 pad).
  - Projection: host transposes init; device computes mask = (init != 0) and
    emb0 = (mask @ col_emb) * cntinv via PE matmuls (bf16 operands, fp32 psum).
  - GCN layer: xl = x @ W (fp32 PE), AllGather xl shards -> xl_full in DRAM,
    then per 128-dest tile: stream 128-edge chunks; for each chunk
    indirect-DMA-gather the 128 source rows (fp32), build the one-hot
    selection matrix sel[slot, dest] = norm * (dest_local == iota) on DVE,
    and accumulate psum += sel.T @ gathered on the PE. Self-loops are
    folded in as extra edges with weight dinv^2. Epilogue adds bias + relu.
  - Attention: softmax over the 2 paths == sigmoid of score difference,
    fused into the second path's layer-3 epilogue.
All normalization factors (degree, rsqrt, edge norms) are host-precomputed.
"""

import os
import sys

sys.path.insert(0, '/opt/trn_rl_repo')

import numpy as np

N_CORES = 8
P = 128


# ----------------------------------------------------------------------------
# Host-side graph preprocessing
# ----------------------------------------------------------------------------

def _prep_graph(edge_index, edge_attr, N, NSH, NSH_PAD):
    """Per-core, per-dest-tile chunked edge lists.

    Returns per core: dloc [TOT,128] f32, norm [TOT,128] f32, gidx [TOT,128] i32
    (padded-global source row ids), plus static per-tile chunk counts
    (max over cores, so one NEFF fits all cores).
    """
    row = np.asarray(edge_index[0], np.int64)
    col = np.asarray(edge_index[1], np.int64)
    w = np.asarray(edge_attr, np.float64)

    deg = np.bincount(col, weights=w, minlength=N) + 1.0
    dinv = 1.0 / np.sqrt(deg)
    norm = dinv[row] * w * dinv[col]

    # append self loops (node n -> n, weight dinv[n]^2)
    self_n = np.arange(N, dtype=np.int64)
    row = np.concatenate([row, self_n])
    col = np.concatenate([col, self_n])
    norm = np.concatenate([norm, dinv * dinv])

    core = col // NSH
    local = col - core * NSH
    tile = local // P
    dloc = local - tile * P
    # padded-global source id for gather into xl_full
    src_pad = (row // NSH) * NSH_PAD + (row % NSH)

    n_tiles = (NSH + P - 1) // P
    per_core = []
    counts = np.zeros((N_CORES, n_tiles), np.int64)
    for c in range(N_CORES):
        m = core == c
        order = np.lexsort((dloc[m], tile[m]))
        t_c = tile[m][order]
        per_core.append((t_c, dloc[m][order], norm[m][order], src_pad[m][order]))
        counts[c] = np.bincount(t_c, minlength=n_tiles)

    chunk_counts = [int(x) for x in np.ceil(counts.max(axis=0) / P).astype(np.int64)]
    tot = int(sum(chunk_counts))
    offs = np.concatenate([[0], np.cumsum(chunk_counts)]).astype(np.int64)

    outs = []
    for c in range(N_CORES):
        t_c, d_c, n_c, s_c = per_core[c]
        dloc_a = np.full((tot, P), 999.0, np.float32)
        norm_a = np.zeros((tot, P), np.float32)
        gidx_a = np.zeros((tot, P), np.int32)
        cnts = np.bincount(t_c, minlength=n_tiles)
        pos = 0
        for t in range(n_tiles):
            k = int(cnts[t])
            sl = slice(pos, pos + k)
            pos += k
            base = offs[t] * P
            fl_d = dloc_a.reshape(-1)
            fl_n = norm_a.reshape(-1)
            fl_g = gidx_a.reshape(-1)
            fl_d[base:base + k] = d_c[sl].astype(np.float32)
            fl_n[base:base + k] = n_c[sl].astype(np.float32)
            fl_g[base:base + k] = s_c[sl].astype(np.int32)
        # device wants [128, TOT] (partition = slot, col = chunk)
        outs.append((np.ascontiguousarray(dloc_a.T), np.ascontiguousarray(norm_a.T),
                     np.ascontiguousarray(gidx_a.T)))
    return outs, chunk_counts, tot


def _host_prep(init, edge_index_scs, edge_attr_scs, edge_index_sls, edge_attr_sls,
               proj_w1, proj_b1, proj_w2, proj_b2, weights, att_w):
    N, M = init.shape
    D = 256
    NSH = N // N_CORES
    n_tiles = (NSH + P - 1) // P
    NSH_PAD = n_tiles * P
    M_PAD = ((M + P - 1) // P) * P

    import ml_dtypes
    np_bf16 = ml_dtypes.bfloat16

    # column embeddings (tiny MLP over column index), fp64 on host then bf16
    cols = np.arange(M, dtype=np.float64)[:, None]
    h = np.maximum(cols @ proj_w1.astype(np.float64) + proj_b1.astype(np.float64), 0.0)
    col_emb = h @ proj_w2.astype(np.float64) + proj_b2.astype(np.float64)   # [M, D]
    col_emb_pad = np.zeros((M_PAD, D), np.float32)
    col_emb_pad[:M] = col_emb.astype(np.float32)
    col_emb_bf = col_emb_pad.astype(np_bf16)

    # init transpose + count of nonzeros per node
    initT = np.ascontiguousarray(init.T)                    # [M, N]
    cnt = np.count_nonzero(init, axis=1).astype(np.float64)  # [N]
    cntinv = np.where(cnt > 0, 1.0 / np.maximum(cnt, 1.0), 0.0).astype(np.float32)

    g_scs, cc_scs, tot_scs = _prep_graph(edge_index_scs, edge_attr_scs, N, NSH, NSH_PAD)
    g_sls, cc_sls, tot_sls = _prep_graph(edge_index_sls, edge_attr_sls, N, NSH, NSH_PAD)

    in_maps = []
    for c in range(N_CORES):
        initT_sh = np.zeros((M_PAD, NSH_PAD), np.float32)
        initT_sh[:M, :NSH] = initT[:, c * NSH:(c + 1) * NSH]
        cinv = np.zeros((P, n_tiles), np.float32)
        src = cntinv[c * NSH:(c + 1) * NSH]
        pad = np.zeros(NSH_PAD, np.float32)
        pad[:NSH] = src
        cinv[:] = pad.reshape(n_tiles, P).T
        d_scs, n_scs, i_scs = g_scs[c]
        d_sls, n_sls, i_sls = g_sls[c]
        im = {
            "initT": initT_sh,
            "col_emb": col_emb_bf,
            "cntinv": cinv,
            "dloc_scs": d_scs, "norm_scs": n_scs, "gidx_scs": i_scs,
            "dloc_sls": d_sls, "norm_sls": n_sls, "gidx_sls": i_sls,
            "att_w": np.ascontiguousarray(
                np.broadcast_to(att_w.astype(np.float32)[None, :], (P, D))),
        }
        for gname in ("scs", "sls"):
            for l in (1, 2, 3):
                W, b = weights[f"{gname}_W{l}"], weights[f"{gname}_b{l}"]
                im[f"W_{gname}{l}"] = np.ascontiguousarray(W.astype(np.float32))
                im[f"b_{gname}{l}"] = np.ascontiguousarray(
                    np.broadcast_to(b.astype(np.float32)[None, :], (P, D)))
        in_maps.append(im)

    meta = dict(N=N, M=M, D=D, NSH=NSH, NSH_PAD=NSH_PAD, M_PAD=M_PAD,
                n_tiles=n_tiles, cc_scs=cc_scs, cc_sls=cc_sls,
                tot_scs=tot_scs, tot_sls=tot_sls)
    return in_maps, meta


# ----------------------------------------------------------------------------
# Device kernel
# ----------------------------------------------------------------------------

def build_nc(meta):
    import concourse.bass as bass
    import concourse.bacc as bacc
    import concourse.mybir as mybir
    import concourse.tile as tile
    from concourse.masks import make_identity

    f32 = mybir.dt.float32
    bf16 = mybir.dt.bfloat16
    i32 = mybir.dt.int32
    Alu = mybir.AluOpType
    Act = mybir.ActivationFunctionType

    D = meta["D"]
    NSH_PAD = meta["NSH_PAD"]
    M_PAD = meta["M_PAD"]
    n_tiles = meta["n_tiles"]
    NFULL = NSH_PAD * N_CORES
    n_k_scene = M_PAD // P

    nc = bacc.Bacc("TRN2", target_bir_lowering=False, debug=False,
                   num_devices=N_CORES)

    # I/O
    initT = nc.dram_tensor("initT", [M_PAD, NSH_PAD], f32, kind="ExternalInput")
    col_emb = nc.dram_tensor("col_emb", [M_PAD, D], bf16, kind="ExternalInput")
    cntinv = nc.dram_tensor("cntinv", [P, n_tiles], f32, kind="ExternalInput")
    att_w = nc.dram_tensor("att_w", [P, D], f32, kind="ExternalInput")
    gin = {}
    for g in ("scs", "sls"):
        tot = meta[f"tot_{g}"]
        gin[g] = dict(
            dloc=nc.dram_tensor(f"dloc_{g}", [P, tot], f32, kind="ExternalInput"),
            norm=nc.dram_tensor(f"norm_{g}", [P, tot], f32, kind="ExternalInput"),
            gidx=nc.dram_tensor(f"gidx_{g}", [P, tot], i32, kind="ExternalInput"),
            W=[nc.dram_tensor(f"W_{g}{l}", [D, D], f32, kind="ExternalInput")
               for l in (1, 2, 3)],
            b=[nc.dram_tensor(f"b_{g}{l}", [P, D], f32, kind="ExternalInput")
               for l in (1, 2, 3)],
        )
    out = nc.dram_tensor("out", [NSH_PAD, D], f32, kind="ExternalOutput")

    with tile.TileContext(nc) as tc:
        with (
            tc.tile_pool(name="const", bufs=1) as cpool,
            tc.tile_pool(name="state", bufs=1) as state,
            tc.tile_pool(name="mask", bufs=3) as mpool,
            tc.tile_pool(name="sel", bufs=6) as selpool,
            tc.tile_pool(name="gath", bufs=8) as gpool,
            tc.tile_pool(name="work", bufs=4) as wpool,
            tc.tile_pool(name="psum", bufs=4, space="PSUM") as pp,
            tc.tile_pool(name="psumT", bufs=4, space="PSUM") as ppT,
            tc.tile_pool(name="dram", bufs=1, space="DRAM") as dram,
        ):
            # ---------------- constants ----------------
            ident = cpool.tile([P, P], f32)
            make_identity(nc, ident[:])
            iota_i = cpool.tile([P, P], i32)
            nc.gpsimd.iota(iota_i[:], pattern=[[1, P]], base=0, channel_multiplier=0)
            iota_f = cpool.tile([P, P], f32)
            nc.vector.tensor_copy(out=iota_f[:], in_=iota_i[:])
            cinv_sb = cpool.tile([P, n_tiles], f32)
            nc.sync.dma_start(out=cinv_sb[:], in_=cntinv[:])
            attw_sb = cpool.tile([P, D], f32)
            nc.sync.dma_start(out=attw_sb[:], in_=att_w[:])
            cemb_sb = cpool.tile([P, n_k_scene, D], bf16)
            nc.sync.dma_start(
                out=cemb_sb[:],
                in_=col_emb.ap().rearrange("(k p) d -> p k d", p=P))

            # persistent node state
            x_cur = state.tile([P, n_tiles, D], f32, tag="x_cur")
            h3_scs = state.tile([P, n_tiles, D], f32, tag="h3_scs")
            sc_scs = state.tile([P, n_tiles], f32, tag="sc_scs")

            emb0_park = dram.tile([NSH_PAD, D], f32)
            xl_shard = dram.tile([NSH_PAD, D], f32)
            xl_full = dram.tile([NFULL, D], f32)

            # ---------------- projection ----------------
            SLAB = 4  # node tiles per pass (bounded by psum "acc" slots)
            for s0 in range(0, n_tiles, SLAB):
                s1 = min(s0 + SLAB, n_tiles)
                width = (s1 - s0) * P
                psums = [pp.tile([P, D], f32, tag="acc", name=f"pj{s0}_{j}")
                         for j in range(s1 - s0)]
                for k in range(n_k_scene):
                    raw = mpool.tile([P, width], f32, tag="rawinit")
                    nc.sync.dma_start(
                        out=raw[:], in_=initT[k * P:(k + 1) * P, s0 * P:s1 * P])
                    mask = mpool.tile([P, width], bf16, tag="mask")
                    nc.vector.tensor_scalar(
                        out=mask[:], in0=raw[:], scalar1=0.0, scalar2=None,
                        op0=Alu.not_equal)
                    for j in range(s1 - s0):
                        nc.tensor.matmul(
                            out=psums[j][:],
                            lhsT=mask[:, j * P:(j + 1) * P],
                            rhs=cemb_sb[:, k, :],
                            start=(k == 0), stop=(k == n_k_scene - 1))
                for j in range(s1 - s0):
                    t = s0 + j
                    nc.scalar.activation(
                        out=x_cur[:, t, :], in_=psums[j][:], func=Act.Copy,
                        scale=cinv_sb[:, t:t + 1])
                    nc.sync.dma_start(
                        out=emb0_park[:].rearrange("(t p) d -> p t d", p=P)[:, t, :],
                        in_=x_cur[:, t, :])

            # ---------------- GCN layers ----------------
            def gcn_layer(g, l, is_last_graph):
                gi = gin[g]
                cc = meta[f"cc_{g}"]
                # W as lhsT chunks: W[k*P:(k+1)*P, :] is [K=P, D]
                W_sb = wpool.tile([P, 2, D], f32, tag="W")
                nc.sync.dma_start(
                    out=W_sb[:], in_=gi["W"][l].ap().rearrange("(k p) d -> p k d", p=P))
                b_sb = wpool.tile([P, D], f32, tag="bias")
                nc.sync.dma_start(out=b_sb[:], in_=gi["b"][l][:])

                # xl = x @ W  (via PE transpose of x tiles)
                for t in range(n_tiles):
                    xT = [ppT.tile([P, P], f32, tag="xT", name=f"xT{t}_{k}")
                          for k in range(2)]
                    for k in range(2):
                        nc.tensor.transpose(
                            out=xT[k][:], in_=x_cur[:, t, k * P:(k + 1) * P],
                            identity=ident[:])
                    xT_sb = wpool.tile([P, 2, P], f32, tag="xT_sb")
                    for k in range(2):
                        nc.vector.tensor_copy(out=xT_sb[:, k, :], in_=xT[k][:])
                    ps = pp.tile([P, D], f32, tag="acc")
                    for k in range(2):
                        nc.tensor.matmul(
                            out=ps[:], lhsT=xT_sb[:, k, :], rhs=W_sb[:, k, :],
                            start=(k == 0), stop=(k == 1))
                    xl_sb = wpool.tile([P, D], f32, tag="xl_sb")
                    nc.vector.tensor_copy(out=xl_sb[:], in_=ps[:])
                    nc.sync.dma_start(
                        out=xl_shard[:].rearrange("(t p) d -> p t d", p=P)[:, t, :],
                        in_=xl_sb[:])

                nc.gpsimd.collective_compute(
                    "AllGather", Alu.bypass,
                    replica_groups=[list(range(N_CORES))],
                    ins=[xl_shard.opt()], outs=[xl_full.opt()])

                # aggregation per dest tile
                col0 = 0
                for t in range(n_tiles):
                    nch = cc[t]
                    ps = pp.tile([P, D], f32, tag="acc")
                    for c in range(nch):
                        col = col0 + c
                        gt = gpool.tile([P, D], f32, tag="g")
                        nc.gpsimd.indirect_dma_start(
                            out=gt[:], out_offset=None, in_=xl_full[:],
                            in_offset=bass.IndirectOffsetOnAxis(
                                ap=idx_sb[:, col:col + 1], axis=0))
                        st = selpool.tile([P, P], f32, tag="sel")
                        nc.vector.scalar_tensor_tensor(
                            out=st[:], in0=iota_f[:],
                            scalar=dloc_sb[:, col:col + 1],
                            in1=norm_sb[:, col:col + 1].to_broadcast([P, P]),
                            op0=Alu.is_equal, op1=Alu.mult)
                        nc.tensor.matmul(
                            out=ps[:], lhsT=st[:], rhs=gt[:],
                            start=(c == 0), stop=(c == nch - 1))
                    col0 += nch

                    if l < 2:
                        tmp = wpool.tile([P, D], f32, tag="epi")
                        nc.vector.tensor_add(out=tmp[:], in0=ps[:], in1=b_sb[:])
                        nc.scalar.activation(
                            out=x_cur[:, t, :], in_=tmp[:], func=Act.Relu)
                    elif not is_last_graph:
                        tmp = wpool.tile([P, D], f32, tag="epi")
                        nc.vector.tensor_add(out=tmp[:], in0=ps[:], in1=b_sb[:])
                        nc.scalar.activation(
                            out=h3_scs[:, t, :], in_=tmp[:], func=Act.Relu)
                        scr = wpool.tile([P, D], f32, tag="scr")
                        nc.vector.scalar_tensor_tensor(
                            out=scr[:], in0=h3_scs[:, t, :], scalar=1.0,
                            in1=attw_sb[:], op0=Alu.mult, op1=Alu.mult,
                            accum_out=sc_scs[:, t:t + 1])
                    else:
                        tmp = wpool.tile([P, D], f32, tag="epi")
                        nc.vector.tensor_add(out=tmp[:], in0=ps[:], in1=b_sb[:])
                        h3b = wpool.tile([P, D], f32, tag="h3b")
                        nc.scalar.activation(out=h3b[:], in_=tmp[:], func=Act.Relu)
                        scb = wpool.tile([P, 1], f32, tag="scb")
                        scr = wpool.tile([P, D], f32, tag="scr")
                        nc.vector.scalar_tensor_tensor(
                            out=scr[:], in0=h3b[:], scalar=1.0,
                            in1=attw_sb[:], op0=Alu.mult, op1=Alu.mult,
                            accum_out=scb[:])
                        dsc = wpool.tile([P, 1], f32, tag="dsc")
                        nc.vector.tensor_sub(
                            out=dsc[:], in0=sc_scs[:, t:t + 1], in1=scb[:])
                        wat = wpool.tile([P, 1], f32, tag="wat")
                        nc.scalar.activation(
                            out=wat[:], in_=dsc[:], func=Act.Sigmoid)
                        dif = wpool.tile([P, D], f32, tag="dif")
                        nc.vector.tensor_sub(
                            out=dif[:], in0=h3_scs[:, t, :], in1=h3b[:])
                        ot = wpool.tile([P, D], f32, tag="ot")
                        nc.vector.scalar_tensor_tensor(
                            out=ot[:], in0=dif[:], scalar=wat[:, 0:1],
                            in1=h3b[:], op0=Alu.mult, op1=Alu.add)
                        nc.sync.dma_start(
                            out=out.ap().rearrange("(t p) d -> p t d", p=P)[:, t, :],
                            in_=ot[:])

            for ig, g in enumerate(("scs", "sls")):
                tot = meta[f"tot_{g}"]
                gi = gin[g]
                idx_sb = cpool.tile([P, tot], i32, tag=f"gidx_{g}")
                dloc_sb = cpool.tile([P, tot], f32, tag=f"dloc_{g}")
                norm_sb = cpool.tile([P, tot], f32, tag=f"norm_{g}")
                nc.sync.dma_start(out=idx_sb[:], in_=gi["gidx"][:])
                nc.sync.dma_start(out=dloc_sb[:], in_=gi["dloc"][:])
                nc.sync.dma_start(out=norm_sb[:], in_=gi["norm"][:])
                if ig == 1:
                    # reset x to emb0
                    nc.sync.dma_start(
                        out=x_cur[:],
                        in_=emb0_park[:].rearrange("(t p) d -> p t d", p=P))
                for l in range(3):
                    gcn_layer(g, l, is_last_graph=(ig == 1))

    nc.finalize()
    return nc


# ----------------------------------------------------------------------------
# Entry point
# ----------------------------------------------------------------------------

def kernel(init, edge_index_scs, edge_attr_scs, edge_index_sls, edge_attr_sls,
           proj_w1, proj_b1, proj_w2, proj_b2,
           scs_W1, scs_b1, scs_W2, scs_b2, scs_W3, scs_b3,
           sls_W1, sls_b1, sls_W2, sls_b2, sls_W3, sls_b3,
           att_w, att_b, _trace=False):
    from concourse import bass_utils

    init = np.asarray(init)
    weights = dict(scs_W1=scs_W1, scs_b1=scs_b1, scs_W2=scs_W2, scs_b2=scs_b2,
                   scs_W3=scs_W3, scs_b3=scs_b3,
                   sls_W1=sls_W1, sls_b1=sls_b1, sls_W2=sls_W2, sls_b2=sls_b2,
                   sls_W3=sls_W3, sls_b3=sls_b3)
    in_maps, meta = _host_prep(
        init, np.asarray(edge_index_scs), np.asarray(edge_attr_scs),
        np.asarray(edge_index_sls), np.asarray(edge_attr_sls),
        np.asarray(proj_w1), np.asarray(proj_b1), np.asarray(proj_w2),
        np.asarray(proj_b2), {k: np.asarray(v) for k, v in weights.items()},
        np.asarray(att_w))
    # att_b cancels inside the 2-way softmax; unused by the device kernel.

    nc = build_nc(meta)
    res = bass_utils.run_bass_kernel_spmd(
        nc, in_maps, core_ids=list(range(N_CORES)), trace=_trace)

    NSH = meta["NSH"]
    out = np.concatenate(
        [np.asarray(res.results[c]["out"])[:NSH] for c in range(N_CORES)], axis=0)
    if _trace:
        return out.astype(np.float32), res
    return out.astype(np.float32)
